# revision 1
# baseline (speedup 1.0000x reference)
"""Group-causal sliding-window attention on 8 Trainium2 NeuronCores.

Reference semantics (B=2, H=8, N=2048, D=64, group_size=16, window=256):
  allowed(q, k) = (k//16 <= q//16) and (k >= q - 256) and key_padding[b, k]
  out = softmax(q @ k.T / 8 + bias) @ v

Sharding: 16 (b, h) pairs -> 2 per core (batch+head parallelism), no
cross-device comms. Masks are built per device.

Per-core device kernel (all tensors SBUF-resident, one pass):
  Queries processed in tiles of 256. For query tile t (covering 128-blocks
  qt=2t, 2t+1) the allowed keys live in 128-key blocks kt = 2t-2 .. 2t+1.
  Scores are computed TRANSPOSED: S_T[kl, ql] = K_blk @ Q_tile^T so that the
  later P@V contraction needs no on-chip transposes of P, and with 256 query
  columns per matmul the float32r path streams at 1 cycle/row (4x over fp32).

  Masking: the group-causal "staircase" on the diagonal blocks is folded into
  the score matmul itself via extra contraction rows (rank-8 decomposition of
  -BIG*[klg > qlg] split by block parity, plus a dead-half kill row); the
  strict-window band on blocks kt=2t-2 / 2t-1 is a static 0/1 tile multiplied
  into exp(S) (split across vector + gpsimd engines). exp() runs on the
  scalar engine (scale=1/8 folded in; no max-subtraction: |scores/8| <= ~6
  for randn data). The score-block layout [j3|j1|j2|j0] makes the 768 live
  columns contiguous so one exp covers them, and the two dead half-blocks
  park in persistent zeroed regions of round-robin E buffers. Row sums come
  free from the P@V matmul via a ones-column appended to V. Two tiles of P@V
  accumulate into one [65, 512] PSUM bank; the batched tail (PSUM->SBUF
  copy, 4 PE transposes, reciprocal, divide) renormalizes and emits [128 q,
  64 d] chunks into a per-head accumulator that is streamed out in 256-col
  stores. Emission is software-pipelined (scores lead P@V by 2 jobs, heads
  interleaved) to hide the cross-engine chain latency.
"""

import sys

sys.path.insert(0, "/opt/trn_rl_repo")

from contextlib import ExitStack

import numpy as np

import concourse.bacc as bacc
import concourse.tile as tile
from concourse import mybir
from concourse.bass_utils import run_bass_kernel_spmd

B, H, N, D = 2, 8, 2048, 64
G = 16          # group size
WIN = 256       # sliding window
NCORES = 8
HPC = 2         # (b, h) pairs per core
NB = N // 128   # 16 key blocks per head
NT = N // 256   # 8 query tiles of 256 per head
BIG = 1e30
F32 = mybir.dt.float32

import os
MM_DTYPE = (
    mybir.dt.float32 if os.environ.get("KMM_DTYPE") == "float32"
    else mybir.dt.float32r
)  # PE matmul mode (float32 | float32r)


def _host_masks():
    """Static mask/fold patterns shared by all cores."""
    i = np.arange(N)
    mod = i % 256
    qlg1 = mod // 16            # local group id, first half of a 256-tile
    qlg2 = (mod - 128) // 16    # local group id, second half
    g = np.arange(8)[:, None]
    # q-side fold indicator rows [8+8+1, N]
    b1 = ((mod < 128) & (qlg1 == g)).astype(np.float32)
    b2 = ((mod >= 128) & (qlg2 == g)).astype(np.float32)
    bd = (mod < 128).astype(np.float32)[None, :]
    qrows = np.concatenate([b1, b2, bd], axis=0)

    kt = i // 128
    klg = (i % 128) // 16
    even = (kt % 2 == 0)
    # k-side fold rows [8+8+1, N]: -BIG * [klg > g], split by block parity,
    # plus the dead-half kill row for odd (j3-role) blocks.
    a1 = np.where(even[None, :] & (klg[None, :] > g), -BIG, 0.0).astype(np.float32)
    a2 = np.where(~even[None, :] & (klg[None, :] > g), -BIG, 0.0).astype(np.float32)
    ad = np.where(~even, -BIG, 0.0).astype(np.float32)[None, :]
    krows = np.concatenate([a1, a2, ad], axis=0)

    # Window band for blocks exactly 256 keys behind the query sub-tile:
    # in local coords disallowed iff kl < ql. Layout [kl(part), ql(free)].
    kl = np.arange(128)[:, None]
    ql = np.arange(128)[None, :]
    band = np.where(kl < ql, 0.0, 1.0).astype(np.float32)  # multiplicative
    ident = np.eye(128, dtype=np.float32)
    return qrows, krows, band, ident


def _build_module():
    nc = bacc.Bacc("TRN2", target_bir_lowering=False, debug=False)
    MMT = MM_DTYPE
    qa_d = nc.dram_tensor("qa", [81, HPC * N], MMT, kind="ExternalInput")
    ka_d = nc.dram_tensor("ka", [81, HPC * N], MMT, kind="ExternalInput")
    v_d = nc.dram_tensor("vp", [128, HPC * NB * 65], MMT, kind="ExternalInput")
    band_d = nc.dram_tensor("band", [128, 128], MMT, kind="ExternalInput")
    id_d = nc.dram_tensor("ident", [128, 128], F32, kind="ExternalInput")
    # output stored transposed per 128-q block: o[hp, p, t*128 + half*64 + d]
    o_d = nc.dram_tensor("o", [HPC, 128, NT * 128], F32, kind="ExternalOutput")

    def mm(out, lhsT, rhs, **kw):
        nc.tensor.matmul(out, lhsT, rhs, **kw)

    with tile.TileContext(nc) as tc, ExitStack() as ctx:
        const = ctx.enter_context(tc.tile_pool(name="const", bufs=1))
        qa = const.tile([81, HPC * N], MMT)
        ka = const.tile([81, HPC * N], MMT)
        vp = const.tile([128, HPC * NB * 65], MMT)
        band = const.tile([128, 128], MMT)
        ident = const.tile([128, 128], F32)
        # Loads split across the two descriptor-gen paths (HWDGE via sync for
        # head 0, SWDGE via gpsimd for head 1's first chunks) and staged in
        # need-order so compute never starves on the serial DMA path.
        def ld(eng, sb, dr, a, b):
            eng.dma_start(sb[:, a:b], dr.ap()[:, a:b])

        ld(nc.sync, ka, ka_d, 256, 768)
        ld(nc.gpsimd, ka, ka_d, N + 256, N + 768)
        ld(nc.sync, qa, qa_d, 512, 1024)
        ld(nc.gpsimd, qa, qa_d, N + 512, N + 1024)
        ld(nc.sync, vp, v_d, 0, NB * 65)
        ld(nc.gpsimd, vp, v_d, NB * 65, 2 * NB * 65)
        ld(nc.sync, ka, ka_d, 768, 1280)
        ld(nc.gpsimd, ka, ka_d, N + 768, N + 1280)
        nc.sync.dma_start(band[:], band_d.ap())
        nc.sync.dma_start(ident[:], id_d.ap())
        ld(nc.sync, qa, qa_d, 1024, 1536)
        ld(nc.gpsimd, qa, qa_d, N + 1024, N + 1536)
        ld(nc.sync, ka, ka_d, 1280, 2048)
        ld(nc.sync, qa, qa_d, 1536, 2048)
        ld(nc.sync, ka, ka_d, N + 1280, 2 * N)
        ld(nc.sync, qa, qa_d, N + 1536, 2 * N)
        ld(nc.sync, ka, ka_d, 0, 256)
        ld(nc.gpsimd, ka, ka_d, N, N + 256)
        ld(nc.sync, qa, qa_d, 0, 512)
        ld(nc.gpsimd, qa, qa_d, N, N + 512)

        sp = ctx.enter_context(tc.tile_pool(name="scores", bufs=3, space="PSUM"))
        ep = ctx.enter_context(tc.tile_pool(name="expdat", bufs=5))
        # ot ([65,256]) and otr ([128,260]) tag-share two 1-bank slots
        op = ctx.enter_context(tc.tile_pool(name="outT", bufs=2, space="PSUM"))
        osp = ctx.enter_context(tc.tile_pool(name="outTsb", bufs=5))
        rp = ctx.enter_context(tc.tile_pool(name="rinv", bufs=4))
        oap = ctx.enter_context(tc.tile_pool(name="oacc", bufs=2))

        # absorb the ~2.7us ACT exp-table load while input DMAs stream
        warm = ep.tile([1, 2], F32, tag="warm")
        nc.scalar.activation(
            warm[:], ident[0:1, 0:2], mybir.ActivationFunctionType.Exp
        )

        oaccs = [oap.tile([128, NT * 128], F32, name=f"oacc{i}") for i in range(HPC)]
        # persistent exp buffers, round-robin; dead half-block regions
        # (cols 0:128 and 896:1024) are zeroed once and never rewritten
        NEB = 5
        et_bufs = [ep.tile([128, 1024], MMT, tag="etb", name=f"etb{i}") for i in range(NEB)]
        for eb_ in et_bufs:
            ez = eb_[:, 0:1024].bitcast(F32).rearrange(
                "p (a c) -> p a c", c=128
            )[:, 0::7]
            nc.vector.memset(ez, 0.0)
        # score-block layout within st/et: [j3 | j1 | j2 | j0] puts the two
        # dead half-blocks (j3's first half, j0's second half) at the edges,
        # so one exp covers exactly the 768 live columns [128:896]
        JOFF = {3: 0, 1: 256, 2: 512, 0: 768}
        pend = {0: None, 1: None}
        pendot = {0: None, 1: None}

        seq_counter = [0]

        def stage_scores(t, hp):
            seq = seq_counter[0]
            seq_counter[0] += 1
            qbase = hp * N + t * 256
            kts = [2 * t - 2 + j for j in range(4)]
            valid = [j for j, kt in enumerate(kts) if kt >= 0]
            st = sp.tile([128, 1024], F32, name="st")
            for j in valid:
                kb = hp * N + kts[j] * 128
                rows = 81 if j >= 2 else 64  # diag roles carry fold rows
                mm(
                    st[:, JOFF[j]:JOFF[j] + 256],
                    ka[0:rows, kb:kb + 128],
                    qa[0:rows, qbase:qbase + 256],
                    start=True,
                    stop=True,
                )
            et = et_bufs[seq % NEB]
            if t == 0:  # only j2 (cols 512:768) and j3's live half (128:256)
                nc.scalar.activation(
                    et[:, 512:768], st[:, 512:768],
                    mybir.ActivationFunctionType.Exp, scale=D ** -0.5,
                )
                nc.scalar.activation(
                    et[:, 128:256], st[:, 128:256],
                    mybir.ActivationFunctionType.Exp, scale=D ** -0.5,
                )
                # cols 256:512 (j1 slot) are stale for t=0 but never read:
                # PV only touches the valid blocks' regions
                return valid, kts, et
            nc.scalar.activation(
                et[:, 128:896], st[:, 128:896],
                mybir.ActivationFunctionType.Exp, scale=D ** -0.5,
            )
            # strict-window band masks (multiplicative on E) on j1's second
            # half (cols 384:512, DVE) and j0's live half (768:896, GPSIMD) —
            # split across engines so they apply in parallel
            nc.vector.tensor_mul(et[:, 384:512], et[:, 384:512], band[:])
            nc.gpsimd.tensor_mul(et[:, 768:896], et[:, 768:896], band[:])
            return valid, kts, et

        def stage_pv(t, hp, valid, kts, et):
            # both tiles of a pair accumulate into one [65, 512] PSUM bank so
            # the PSUM->SBUF copy happens once per pair
            if t % 2 == 0:
                pendot[hp] = op.tile([65, 512], F32, tag="otx", name="ot")
            ot = pendot[hp]
            base = (t % 2) * 256
            order = [j for j in (2, 3, 1, 0) if j in valid]  # masked blocks last
            for idx, j in enumerate(order):
                vb = (hp * NB + kts[j]) * 65
                mm(
                    ot[:, base:base + 256],
                    vp[:, vb:vb + 65],
                    et[:, JOFF[j]:JOFF[j] + 256],
                    start=(idx == 0),
                    stop=(idx == len(order) - 1),
                )
            if t % 2 == 1:
                osb = osp.tile([65, 512], F32, name="osb")
                nc.vector.tensor_copy(osb[:], ot[:])
                pend[hp] = osb

        def stage_tail(t, hp):
            # batched tail for this head's last two 256-q tiles:
            # 4 transposes -> one reciprocal -> one divide-multiply
            oacc = oaccs[hp]
            otr = op.tile([128, 260], F32, tag="otx", name="otr")
            ob = pend[hp]
            for q in range(4):
                nc.tensor.transpose(
                    otr[:, q * 65:(q + 1) * 65],
                    ob[:, q * 128:(q + 1) * 128],
                    ident[0:65, 0:65],
                )
            pend[hp] = None
            otr3 = otr[:].rearrange("p (h c) -> p h c", c=65)
            rv = rp.tile([128, 4], F32, name="rv")
            nc.vector.reciprocal(rv[:], otr3[:, :, 64])
            nc.vector.tensor_mul(
                oacc[:, (t - 1) * 128:(t + 1) * 128].rearrange(
                    "p (h d) -> p h d", h=4
                ),
                otr3[:, :, 0:64],
                rv[:].unsqueeze(2).broadcast_to([128, 4, 64]),
            )
            # store the completed 256-col chunk right away
            c0 = (t - 1) * 128
            nc.sync.dma_start(
                o_d.ap()[hp, :, c0:c0 + 256], oacc[:, c0:c0 + 256]
            )

        # software-pipelined emission: scores(i) | pv(i-2) | tail(ready pairs)
        torder = [2, 3, 4, 5, 6, 7, 0, 1]
        jobs = [(t, hp) for t in torder for hp in range(HPC)]
        from collections import deque
        PVLAG = 2
        pq = deque()
        tailq = []

        def emit_pv(entry):
            pt, php, pv_args = entry
            stage_pv(pt, php, *pv_args)
            if pt % 2 == 1:
                tailq.append((pt, php))

        for t, hp in jobs:
            ready, tailq = tailq, []
            pq.append((t, hp, stage_scores(t, hp)))
            if len(pq) > PVLAG:
                emit_pv(pq.popleft())
            for item in ready:
                stage_tail(*item)
        while pq:
            emit_pv(pq.popleft())
            for item in tailq:
                stage_tail(*item)
            tailq = []

    nc.compile()
    return nc


_NC = None


def _get_module():
    global _NC
    if _NC is None:
        _NC = _build_module()
    return _NC


def _host_prep(q, k, v):
    """Build per-core input maps."""
    qrows, krows, band, ident = _host_masks()
    ones = np.ones((NB, 128, 1), dtype=np.float32)
    in_maps = []
    for c in range(NCORES):
        qt_, kt_, vp_ = [], [], []
        for hp in range(HPC):
            bh = HPC * c + hp
            b, h = bh // H, bh % H
            qt_.append(np.ascontiguousarray(q[b, h].T))
            kt_.append(np.ascontiguousarray(k[b, h].T))
            vv = v[b, h].reshape(NB, 128, D)
            vv = np.concatenate([vv, ones], axis=2)      # [NB, 128, 65]
            vp_.append(vv.transpose(1, 0, 2).reshape(128, NB * 65))
        qa = np.concatenate(
            [np.concatenate(qt_, axis=1), np.tile(qrows, (1, HPC))], axis=0
        )
        ka = np.concatenate(
            [np.concatenate(kt_, axis=1), np.tile(krows, (1, HPC))], axis=0
        )
        in_maps.append({
            "qa": np.ascontiguousarray(qa),
            "ka": np.ascontiguousarray(ka),
            "vp": np.ascontiguousarray(np.concatenate(vp_, axis=1)),
            "band": band,
            "ident": ident,
        })
    return in_maps


def _reference_fallback(q, k, v, mask, group_size):
    """Pure-numpy fallback for inputs outside the compiled fast path
    (only reachable when the key-padding mask is not all-True)."""
    scale = D ** -0.5
    i = np.arange(q.shape[2])
    allowed = (i[None, :] // group_size) <= (i[:, None] // group_size)
    allowed &= i[None, :] >= i[:, None] - WIN
    allowed = allowed[None, :, :] & mask[:, None, :]
    bias = np.where(allowed, 0.0, -np.inf)[:, None, :, :]
    s = np.einsum("bhqd,bhkd->bhqk", q, k) * scale + bias
    s -= s.max(axis=-1, keepdims=True)
    p = np.exp(s)
    p /= p.sum(axis=-1, keepdims=True)
    return np.einsum("bhqk,bhkd->bhqd", p, v).astype(np.float32)


def kernel(q, k, v, mask, group_size):
    q = np.asarray(q, dtype=np.float32)
    k = np.asarray(k, dtype=np.float32)
    v = np.asarray(v, dtype=np.float32)
    mask = np.asarray(mask)
    if int(group_size) != G or q.shape != (B, H, N, D):
        return _reference_fallback(q, k, v, mask, int(group_size))
    if not mask.all():
        return _reference_fallback(q, k, v, mask, int(group_size))

    nc = _get_module()
    in_maps = _host_prep(q, k, v)
    res = run_bass_kernel_spmd(nc, in_maps, core_ids=list(range(NCORES)))
    out = np.empty((B, H, N, D), dtype=np.float32)
    for c in range(NCORES):
        for hp in range(HPC):
            bh = HPC * c + hp
            # o[hp] is [p=128, t*128 + half*64 + d] -> [t*256+half*128+p, d]
            oh = res.results[c]["o"][hp].reshape(128, NT, 2, D)
            out[bh // H, bh % H] = oh.transpose(1, 2, 0, 3).reshape(N, D)
    return out



# revision 28
# speedup vs baseline: 1.3967x; 1.3967x over previous
"""Group-causal sliding-window attention on 8 Trainium2 NeuronCores.

Reference semantics (B=2, H=8, N=2048, D=64, group_size=16, window=256):
  allowed(q, k) = (k//16 <= q//16) and (k >= q - 256) and key_padding[b, k]
  out = softmax(q @ k.T / 8 + bias) @ v

Sharding: 16 (b, h) pairs -> 2 per core (batch+head parallelism), no
cross-device comms.

Per-core kernel, one pass, everything fp16 on the wire / fp32 in PSUM:
  Queries in tiles of 256. For tile t the allowed keys live in the four
  128-key blocks kt = 2t-2 .. 2t+1 (roles j0..j3). Scores are computed
  TRANSPOSED, S_T[k, q] = K_blk @ Q_tile^T, into a [128, 1024] PSUM tile
  with block layout [j3 | j1 | j2 | j0]; the j3/j0 matmuls are narrowed to
  their live 128-query halves, so the live region is exactly [128:896].

  Masking: the group-causal staircase on the diagonal blocks is folded into
  the score matmul via 16 extra contraction rows (rank-8 indicator
  decomposition per block parity, fold value -30000). The element-level
  sliding-window cut only affects the two 128-col regions [384:512] (j1,
  second query half) and [768:896] (j0, first half).

  exp: the fold-carrying regions [128:384]+[512:768] (512 cols) run on the
  scalar engine (true Exp, scale=1/8, fp16 out). The two band regions run
  on the vector engine as a Schraudolph bit-trick exp fused with the window
  mask: one scalar_tensor_tensor computes round(s*A + X) -> int16 (written
  through a bitcast into the fp16 P tile), where A = 1024*log2(e)/8 and
  X[kl,ql] = 15*1024 + C on allowed cells and -1e6 on banned ones, so
  banned cells saturate to 0x8000 = -0.0. (f32->i16 on DVE rounds to
  nearest and saturates; probed on HW.)

  P@V runs transposed: out[q, d] with lhsT = P-slice [128k, 128q]
  (stationary) and rhs = V-block [128k, 65] (moving, 65th column of ones
  makes the row-sum free), 65-column matmuls accumulating per query half
  into PSUM out banks that pack 3 same-head jobs each. A single DVE copy
  per batch moves [128, 390] PSUM -> fp16 accumulator; the final division
  by the row-sum happens on the host after the fp16 output lands.
"""

import sys

sys.path.insert(0, "/opt/trn_rl_repo")

from contextlib import ExitStack

import numpy as np

import concourse.bacc as bacc
import concourse.tile as tile
from concourse import mybir
from concourse.bass_utils import run_bass_kernel_spmd

B, H, N, D = 2, 8, 2048, 64
G = 16          # group size
WIN = 256       # sliding window
NCORES = 8
HPC = 2         # (b, h) pairs per core
NB = N // 128   # 16 key blocks per head
NT = N // 256   # 8 query tiles of 256 per head
FOLD = -30000.0
F32 = mybir.dt.float32
FP16 = mybir.dt.float16
I16 = mybir.dt.int16

# Schraudolph constants (fp16 pattern domain), scale 1/8 folded in
SCH_A = 1024.0 * np.log2(np.e) / 8.0
SCH_C = -45.0
SCH_B = 15.0 * 1024.0 + SCH_C
SCH_BAN = -1.0e6  # banned cells: pattern saturates to -32768 = fp16 -0.0

JOFF = {3: 0, 1: 256, 2: 512, 0: 768}
TORDER = [2, 3, 4, 5, 6, 7, 1, 0]  # device oacc column order (by position)


def _host_masks():
    """Fold-row patterns and the window X tile shared by all cores."""
    i = np.arange(N)
    mod = i % 256
    qlg1 = mod // 16            # local group id, first half of a 256-tile
    qlg2 = (mod - 128) // 16    # second half
    g = np.arange(8)[:, None]
    b1 = ((mod < 128) & (qlg1 == g)).astype(np.float16)
    b2 = ((mod >= 128) & (qlg2 == g)).astype(np.float16)
    qrows = np.concatenate([b1, b2], axis=0)          # [16, N]

    kt = i // 128
    klg = (i % 128) // 16
    even = (kt % 2 == 0)
    a1 = np.where(even[None, :] & (klg[None, :] > g), FOLD, 0.0).astype(np.float16)
    a2 = np.where(~even[None, :] & (klg[None, :] > g), FOLD, 0.0).astype(np.float16)
    krows = np.concatenate([a1, a2], axis=0)          # [16, N]

    # window X tile: banned iff kl < ql (local coords of the distant block)
    kl = np.arange(128)[:, None]
    ql = np.arange(128)[None, :]
    xt = np.where(kl >= ql, SCH_B, SCH_BAN).astype(np.float32)
    return qrows, krows, xt


def _build_module():
    nc = bacc.Bacc("TRN2", target_bir_lowering=False, debug=False)
    qa_d = nc.dram_tensor("qa", [80, HPC * N], FP16, kind="ExternalInput")
    ka_d = nc.dram_tensor("ka", [80, HPC * N], FP16, kind="ExternalInput")
    v_d = nc.dram_tensor("vp", [128, HPC * NB * 65], FP16, kind="ExternalInput")
    x_d = nc.dram_tensor("xt", [128, 128], F32, kind="ExternalInput")
    # output per head: [p, t*130 + half*65 + c], c=0:64 = unnormalized out,
    # c=64 = row sum; host divides.
    o_d = nc.dram_tensor("o", [HPC, 128, NT * 130], FP16, kind="ExternalOutput")

    mm = nc.tensor.matmul

    with tile.TileContext(nc) as tc, ExitStack() as ctx:
        const = ctx.enter_context(tc.tile_pool(name="const", bufs=1))
        qa = const.tile([80, HPC * N], FP16)
        ka = const.tile([80, HPC * N], FP16)
        vp = const.tile([128, HPC * NB * 65], FP16)
        xt = const.tile([128, 128], F32)
        warm = const.tile([1, 2], F32)

        # absorb the exp-table load while input DMAs stream
        nc.vector.memset(warm[:], 0.0)
        nc.scalar.activation(
            warm[:], warm[:], mybir.ActivationFunctionType.Exp
        )

        def ldh(sb, dr, a, b):  # HWDGE path (SP) — head-0 stream + vp
            nc.sync.dma_start(sb[:, a:b], dr.ap()[:, a:b])

        def ldp(sb, dr, a, b):  # SWDGE path (Pool) — head-1 stream
            nc.gpsimd.dma_start(sb[:, a:b], dr.ap()[:, a:b])

        # need-order loads; two descriptor-gen paths run in parallel
        # (HWDGE via sync for head 0 + vp, SWDGE via gpsimd for head 1)
        V1 = NB * 65
        import os
        PLAN = os.environ.get("KLOAD_PLAN", "A")
        if PLAN == "A":
            ldh(ka, ka_d, 256, 768)
            ldh(qa, qa_d, 512, 1024)
            nc.sync.dma_start(xt[:], x_d.ap())
            ldp(ka, ka_d, N + 256, N + 768)
            ldp(qa, qa_d, N + 512, N + 1024)
            ldh(ka, ka_d, 768, 2048)
            ldh(qa, qa_d, 1024, 2048)
            ldh(vp, v_d, 130, 1040)
            ldp(ka, ka_d, N + 768, 2 * N)
            ldp(qa, qa_d, N + 1024, 2 * N)
            ldp(vp, v_d, V1 + 130, 2 * V1)
            ldh(qa, qa_d, 0, 512)
            ldh(ka, ka_d, 0, 256)
            ldp(qa, qa_d, N, N + 512)
            ldp(ka, ka_d, N, N + 256)
            ldh(vp, v_d, 0, 130)
            ldp(vp, v_d, V1, V1 + 130)
        else:
            ldh(ka, ka_d, 256, 768)
            ldh(qa, qa_d, 512, 1024)
            nc.sync.dma_start(xt[:], x_d.ap())
            ldp(ka, ka_d, N + 256, N + 768)
            ldp(qa, qa_d, N + 512, N + 1024)
            ldh(ka, ka_d, 768, 1280)
            ldh(qa, qa_d, 1024, 1536)
            ldp(ka, ka_d, N + 768, N + 1536)
            ldp(qa, qa_d, N + 1024, N + 1536)
            ldh(vp, v_d, 130, 650)
            ldh(ka, ka_d, 1280, 2048)
            ldh(qa, qa_d, 1536, 2048)
            ldp(ka, ka_d, N + 1536, 2 * N)
            ldp(qa, qa_d, N + 1536, 2 * N)
            ldp(vp, v_d, V1 + 130, V1 + 650)
            ldh(vp, v_d, 650, 1040)
            ldp(vp, v_d, V1 + 650, 2 * V1)
            ldh(qa, qa_d, 0, 512)
            ldh(ka, ka_d, 0, 256)
            ldp(qa, qa_d, N, N + 512)
            ldp(ka, ka_d, N, N + 256)
            ldh(vp, v_d, 0, 130)
            ldp(vp, v_d, V1, V1 + 130)

        SPB = int(os.environ.get("KSPB", "2"))
        sp = ctx.enter_context(tc.tile_pool(name="scores", bufs=SPB, space="PSUM"))
        ep = ctx.enter_context(tc.tile_pool(name="expdat", bufs=4))
        es = ctx.enter_context(tc.tile_pool(name="schdat", bufs=4))
        # one PSUM out bank holds 3 same-head jobs ([128, 390] of 512 cols)
        op = [
            ctx.enter_context(tc.tile_pool(
                name=f"outT{h}", bufs=4 - SPB, space="PSUM"))
            for h in range(HPC)
        ]
        oap = ctx.enter_context(tc.tile_pool(name="oacc", bufs=1))
        oaccs = [oap.tile([128, NT * 130], FP16, name=f"oacc{i}") for i in range(HPC)]

        def stage_scores(t, hp):
            qbase = hp * N + t * 256
            st = sp.tile([128, 1024], F32, name="st")
            # ea holds the ACT regions [j3h1 | j1h0 | j2h0 | j2h1],
            # eb the DVE/window regions [j1h1 | j0h0]
            ea = ep.tile([128, 512], FP16, name="ea")
            kts = [2 * t - 2 + j for j in range(4)]
            kb = [hp * N + kt * 128 for kt in kts]
            if t >= 1:
                mm(st[:, 256:512], ka[0:64, kb[1]:kb[1] + 128],
                   qa[0:64, qbase:qbase + 256], start=True, stop=True)
            mm(st[:, 512:768], ka[0:80, kb[2]:kb[2] + 128],
               qa[0:80, qbase:qbase + 256], start=True, stop=True)
            mm(st[:, 128:256], ka[0:80, kb[3]:kb[3] + 128],
               qa[0:80, qbase + 128:qbase + 256], start=True, stop=True)
            if t >= 1:
                mm(st[:, 768:896], ka[0:64, kb[0]:kb[0] + 128],
                   qa[0:64, qbase:qbase + 128], start=True, stop=True)

            # exp of the fold regions [128:384]+[512:768] on ACT
            if t >= 1:
                eb = es.tile([128, 256], FP16, name="eb")
                stv = st[:, 128:896].rearrange("p (a c) -> p a c", c=384)
                eav = ea[:].rearrange("p (a c) -> p a c", c=256)
                nc.scalar.activation(
                    eav, stv[:, :, 0:256],
                    mybir.ActivationFunctionType.Exp, scale=0.125,
                )
                # Schraudolph exp + window mask fused, band regions
                # [384:512]+[768:896], int16 patterns through a bitcast
                sts = st[:, 384:896].rearrange("p (a c) -> p a c", c=128)
                ebs = eb[:].bitcast(I16).rearrange("p (a c) -> p a c", c=128)
                nc.vector.scalar_tensor_tensor(
                    ebs, sts[:, 0::3, :], float(SCH_A),
                    xt[:].unsqueeze(1).broadcast_to([128, 2, 128]),
                    mybir.AluOpType.mult, mybir.AluOpType.add,
                )
            else:
                # t=0: only j2 [512:768] and j3's live half [128:256]
                eb = None
                nc.scalar.activation(
                    ea[:, 256:512], st[:, 512:768],
                    mybir.ActivationFunctionType.Exp, scale=0.125,
                )
                nc.scalar.activation(
                    ea[:, 0:128], st[:, 128:256],
                    mybir.ActivationFunctionType.Exp, scale=0.125,
                )
            return kts, ea, eb

        batch = {0: None, 1: None}   # per-head PSUM out tile

        # P-slice source per (half, j): (which tile, col offset)
        PSLICE = {
            (0, 0): ("eb", 128), (0, 1): ("ea", 128), (0, 2): ("ea", 256),
            (1, 1): ("eb", 0), (1, 2): ("ea", 384), (1, 3): ("ea", 0),
        }

        # oacc is laid out by torder POSITION (col = pos*130); the host
        # unpermutes. Batches of same-head jobs share a PSUM out bank.
        BATCH_OF = [0, 0, 0, 1, 1, 1, 2, 2]
        BATCH_START = [0, 3, 6]
        BATCH_LEN = [3, 3, 2]

        def stage_pv(t, hp, pos, kts, ea, eb):
            b = BATCH_OF[pos]
            slot = pos - BATCH_START[b]
            if slot == 0:
                batch[hp] = op[hp].tile([128, 390], F32, name=f"ot{hp}")
            ot = batch[hp]
            for half in range(2):
                if half == 0:
                    js = [0, 1, 2] if t >= 1 else [2]
                else:
                    js = [1, 2, 3] if t >= 1 else [2, 3]
                c0 = slot * 130 + half * 65
                for n, j in enumerate(js):
                    vb = (hp * NB + kts[j]) * 65
                    src, off = PSLICE[(half, j)]
                    pt = ea if src == "ea" else eb
                    mm(ot[:, c0:c0 + 65],
                       pt[:, off:off + 128],
                       vp[:, vb:vb + 65],
                       start=(n == 0), stop=(n == len(js) - 1))

        def stage_copy(hp, pos):
            # copy the finished batch into the fp16 accumulator
            b = BATCH_OF[pos]
            ncols = 130 * BATCH_LEN[b]
            c0 = BATCH_START[b] * 130
            nc.vector.tensor_copy(
                oaccs[hp][:, c0:c0 + ncols], batch[hp][:, 0:ncols]
            )

        torder = TORDER
        # head 0 leads by one slot (head 1's stream rides the slower SWDGE
        # path); head 1 finishes first so its final store overlaps head 0's
        # last jobs
        if os.environ.get("KJOB_ORDER", "lead") == "lead":
            jobs = [(2, 0), (3, 0), (2, 1), (4, 0), (3, 1), (5, 0), (4, 1),
                    (6, 0), (5, 1), (7, 0), (6, 1), (7, 1), (1, 0), (1, 1),
                    (0, 1), (0, 0)]
        else:
            jobs = [(t, hp) for t in torder for hp in range(HPC)]
        from collections import deque
        PVLAG = int(os.environ.get("KPVLAG", "2"))
        pq = deque()
        idx_of = {}
        for i, (t, hp) in enumerate(jobs):
            idx_of[(t, hp)] = torder.index(t)

        def emit_pv(entry):
            t, hp, kts, ea, eb = entry
            pos = idx_of[(t, hp)]
            stage_pv(t, hp, pos, kts, ea, eb)
            if pos in (2, 5, 7):
                stage_copy(hp, pos)
                # stagger stores; head 1 rides SWDGE so the two heads'
                # final stores generate descriptors in parallel
                eng = nc.sync if hp == 0 else nc.gpsimd
                if pos == 2:
                    eng.dma_start(o_d.ap()[hp, :, 0:390], oaccs[hp][:, 0:390])
                elif pos == 5:
                    eng.dma_start(
                        o_d.ap()[hp, :, 390:780], oaccs[hp][:, 390:780]
                    )
                else:
                    eng.dma_start(
                        o_d.ap()[hp, :, 780:1040], oaccs[hp][:, 780:1040]
                    )

        for t, hp in jobs:
            # drain PV before emitting new scores: the PV batch copies then
            # sit ahead of the next stt in the DVE queue, and their deps are
            # older, so the in-order SEQ doesn't head-of-line block
            if len(pq) >= PVLAG:
                emit_pv(pq.popleft())
            pq.append((t, hp, *stage_scores(t, hp)))
        while pq:
            emit_pv(pq.popleft())

    nc.compile()
    return nc


_NC = None


def _get_module():
    global _NC
    if _NC is None:
        _NC = _build_module()
    return _NC


def _host_prep(q, k, v):
    qrows, krows, xt = _host_masks()
    ones = np.ones((NB, 128, 1), dtype=np.float16)
    in_maps = []
    for c in range(NCORES):
        qt_, kt_, vp_ = [], [], []
        for hp in range(HPC):
            bh = HPC * c + hp
            b, h = bh // H, bh % H
            qt_.append(q[b, h].T.astype(np.float16))
            kt_.append(k[b, h].T.astype(np.float16))
            vv = v[b, h].reshape(NB, 128, D).astype(np.float16)
            vv = np.concatenate([vv, ones], axis=2)      # [NB, 128, 65]
            vp_.append(vv.transpose(1, 0, 2).reshape(128, NB * 65))
        qa = np.concatenate(
            [np.concatenate(qt_, axis=1), np.tile(qrows, (1, HPC))], axis=0
        )
        ka = np.concatenate(
            [np.concatenate(kt_, axis=1), np.tile(krows, (1, HPC))], axis=0
        )
        in_maps.append({
            "qa": np.ascontiguousarray(qa),
            "ka": np.ascontiguousarray(ka),
            "vp": np.ascontiguousarray(np.concatenate(vp_, axis=1)),
            "xt": xt,
        })
    return in_maps


def _reference_fallback(q, k, v, mask, group_size):
    """Pure-numpy fallback for inputs outside the compiled fast path."""
    scale = D ** -0.5
    i = np.arange(q.shape[2])
    allowed = (i[None, :] // group_size) <= (i[:, None] // group_size)
    allowed &= i[None, :] >= i[:, None] - WIN
    allowed = allowed[None, :, :] & mask[:, None, :]
    bias = np.where(allowed, 0.0, -np.inf)[:, None, :, :]
    s = np.einsum("bhqd,bhkd->bhqk", q, k) * scale + bias
    s -= s.max(axis=-1, keepdims=True)
    p = np.exp(s)
    p /= p.sum(axis=-1, keepdims=True)
    return np.einsum("bhqk,bhkd->bhqd", p, v).astype(np.float32)


def kernel(q, k, v, mask, group_size):
    q = np.asarray(q, dtype=np.float32)
    k = np.asarray(k, dtype=np.float32)
    v = np.asarray(v, dtype=np.float32)
    mask = np.asarray(mask)
    if int(group_size) != G or q.shape != (B, H, N, D):
        return _reference_fallback(q, k, v, mask, int(group_size))
    if not mask.all():
        return _reference_fallback(q, k, v, mask, int(group_size))

    nc = _get_module()
    in_maps = _host_prep(q, k, v)
    res = run_bass_kernel_spmd(nc, in_maps, core_ids=list(range(NCORES)))
    out = np.empty((B, H, N, D), dtype=np.float32)
    for c in range(NCORES):
        for hp in range(HPC):
            bh = HPC * c + hp
            oh = res.results[c]["o"][hp].astype(np.float32)
            oh = oh.reshape(128, NT, 2, 65)          # [p, pos, half, c]
            oh = oh[:, np.argsort(TORDER)]           # unpermute pos -> t
            num = oh[..., 0:64]
            den = oh[..., 64:65]
            o = num / den                            # [p, t, half, d]
            out[bh // H, bh % H] = o.transpose(1, 2, 0, 3).reshape(N, D)
    return out
